# revision 2
# baseline (speedup 1.0000x reference)
"""CapsuleLayer dynamic-routing kernel for 8 Trainium2 NeuronCores.

Problem: x [64,2048,16], route_weights [32,2048,16,32] ->
  3-iteration routing -> out [32,64,1,1,32] (fp32).

Sharding: capsules (C=32) split 4-per-core across 8 cores; x replicated.
Per core everything is dense matmuls + DVE elementwise:

  priors[c,b,r,o] = sum_j x[b,r,j] W[c,r,j,o]
  s1 = mean_r priors              -> one big PE contraction over (j,r)
  V_i[c,b,(r,j)] = sum_o W[c,r,j,o] out_i[c,b,o]   (PE, K=o=32, 4-way packed)
  d_i[c,b,r] = sum_j x[b,(r,j)] V_i[c,b,(r,j)]     (DVE mult + grouped reduce)
  logits += d_i ; e = exp(logits - max)            (ACT)
  xe[c][(j,r),b] = xt2[(j,r),b] * eT[c][r,b]       (DVE; j-blocked layout)
  s_{i+1}[c,b,o] = (sum_{(j,r)} xe W) / Z          (PE, K=(j,r))
  out_i = squash(s_i)
"""
import os
import numpy as np

C, B, R, CIN, OUT = 32, 64, 2048, 16, 32
NCORES = 8
CLOC = C // NCORES          # 4 capsules per core
RJ = R * CIN                # 32768
NK = RJ // 128              # 256 chunks of 128 along (j,r) / (r,j)

_CACHE = {}


def _build_program():
    from contextlib import ExitStack
    import concourse.bass as bass
    import concourse.bacc as bacc
    import concourse.tile as tile
    from concourse import mybir

    f32 = mybir.dt.float32
    AL = mybir.AluOpType
    AF = mybir.ActivationFunctionType
    AX = mybir.AxisListType

    nc = bacc.Bacc(None, target_bir_lowering=False,
                   detect_race_conditions=not bool(int(os.environ.get("CAPS_NO_RACE", "0"))))
    n_loops = int(os.environ.get("CAPS_LOOPS", "1"))
    v32r = bool(int(os.environ.get("CAPS_V32R", "0")))
    vsp3 = bool(int(os.environ.get("CAPS_VSPLIT3", "0")))
    f32r = mybir.dt.float32r
    bf16 = mybir.dt.bfloat16

    # ---- DRAM I/O ----
    w2cat = nc.dram_tensor("w2cat", [RJ, 128], f32, kind="ExternalInput")     # [(j,r),(c,o)]
    xt2 = nc.dram_tensor("xt2", [RJ, B], f32, kind="ExternalInput")           # [(j,r),b]
    wt = nc.dram_tensor("wt", [CLOC, OUT, RJ], f32, kind="ExternalInput")     # [c,o,(r,j)]
    if vsp3:
        wth = nc.dram_tensor("wth", [CLOC, OUT, RJ], bf16, kind="ExternalInput")
        wtl = nc.dram_tensor("wtl", [CLOC, OUT, RJ], bf16, kind="ExternalInput")
    x2d = nc.dram_tensor("x2d", [128, RJ], f32, kind="ExternalInput")         # [(2,b),(r,j)]
    ident = nc.dram_tensor("ident", [128, 128], f32, kind="ExternalInput")
    out3 = nc.dram_tensor("out3", [B, 128], f32, kind="ExternalOutput")       # [b,(c,o)]

    with tile.TileContext(nc) as tc, ExitStack() as ctx:
        const = ctx.enter_context(tc.tile_pool(name="const", bufs=1))
        small = ctx.enter_context(tc.tile_pool(name="small", bufs=3))
        wcat_p = ctx.enter_context(tc.tile_pool(name="wcat", bufs=4))
        wt_p = ctx.enter_context(tc.tile_pool(name="wtp", bufs=3))
        big = ctx.enter_context(tc.tile_pool(name="big", bufs=2))
        xe_p = ctx.enter_context(tc.tile_pool(name="xep", bufs=2))
        psacc_p = ctx.enter_context(tc.tile_pool(name="psacc", bufs=4, space="PSUM"))
        psV_p = ctx.enter_context(tc.tile_pool(name="psV", bufs=4, space="PSUM"))
        psT_p = psV_p

        idn = const.tile([128, 128], f32, tag="ident", name="idn")
        nc.sync.dma_start(out=idn, in_=ident[:])

        # resident xt2: [128, (k=256, b=64)]
        xt2_sb = const.tile([128, NK, B], f32, tag="xt2sb", name="xt2_sb")
        nc.sync.dma_start(out=xt2_sb, in_=xt2[:].rearrange("(k p) b -> p k b", p=128))

        # logits per capsule-pair [(2c,b)=128, r=2048]
        lP = [const.tile([128, R], f32, tag=f"l{p}", name=f"lP{p}") for p in range(2)]
        # transposed-unnormalized-probs  [128=r%128, (c=4, rb=16, b=64)]
        p2T = const.tile([128, CLOC, R // 128, B], f32, tag="p2T", name="p2T")

        def squash(u_bT, scale_pow):
            """u_bT [64,(4c,32o)]: s = u*scale_pow; out = s*sqrt(n2)/(n2+1)."""
            sq = small.tile([B, 128], f32, tag="sq", name="sq")
            nc.vector.scalar_tensor_tensor(
                out=sq, in0=u_bT, scalar=float(scale_pow * scale_pow),
                in1=u_bT, op0=AL.mult, op1=AL.mult)
            n2 = small.tile([B, CLOC], f32, tag="n2", name="n2")
            nc.vector.tensor_reduce(
                out=n2, in_=sq[:].rearrange("b (c o) -> b c o", c=CLOC),
                axis=AX.X, op=AL.add)
            rt = small.tile([B, CLOC], f32, tag="rt", name="rt")
            nc.scalar.activation(out=rt, in_=n2, func=AF.Sqrt)
            dn = small.tile([B, CLOC], f32, tag="dn", name="dn")
            nc.vector.tensor_scalar_add(out=dn, in0=n2, scalar1=1.0)
            rc = small.tile([B, CLOC], f32, tag="rc", name="rc")
            nc.vector.reciprocal(out=rc, in_=dn)
            f = small.tile([B, CLOC], f32, tag="f", name="f")
            nc.vector.tensor_mul(out=f, in0=rt, in1=rc)
            f2 = small.tile([B, CLOC], f32, tag="f2", name="f2")
            nc.vector.tensor_scalar_mul(out=f2, in0=f, scalar1=float(scale_pow))
            o_i = small.tile([B, 128], f32, tag="oi", name="oi")
            f2b = bass.AP(tensor=f2[:].tensor, offset=f2[:].offset,
                          ap=[f2[:].ap[0], f2[:].ap[1], [0, OUT]])
            nc.vector.tensor_tensor(
                out=o_i[:].rearrange("b (c o) -> b c o", c=CLOC),
                in0=u_bT[:].rearrange("b (c o) -> b c o", c=CLOC),
                in1=f2b, op=AL.mult)
            psOT = psT_p.tile([128, B], f32, tag="psVT", name="psOT")
            nc.tensor.transpose(psOT, o_i, idn[0:B, 0:B])
            oT = small.tile([128, B], f32, tag="oT", name="oT")
            nc.scalar.copy(out=oT, in_=psOT)
            if v32r:
                oTr = small.tile([128, B], f32r, tag="oTr", name="oTr")
                nc.gpsimd.dma_start(out=oTr, in_=oT)
                return o_i, oTr
            if vsp3:
                oTh = small.tile([128, B], bf16, tag="oTh", name="oTh")
                nc.vector.tensor_copy(out=oTh, in_=oT)
                dfh = small.tile([128, B], f32, tag="dfh", name="dfh")
                nc.vector.tensor_sub(out=dfh, in0=oT, in1=oTh)
                oTl = small.tile([128, B], bf16, tag="oTl", name="oTl")
                nc.vector.tensor_copy(out=oTl, in_=dfh)
                return o_i, (oTh, oTl)
            return o_i, oT

        for _loop in range(n_loops):
            # ---------- Phase A: s1 = (1/R) sum_(j,r) x W ----------
            psA = psacc_p.tile([128, B], f32, tag="acc", name="psA")
            for k in range(NK):
                wck = wcat_p.tile([128, 128], f32, tag="wck", name="wck")
                nc.sync.dma_start(out=wck, in_=w2cat[128 * k:128 * (k + 1), :])
                nc.tensor.matmul(psA, wck, xt2_sb[:, k, :],
                                 start=(k == 0), stop=(k == NK - 1))
            sA = small.tile([128, B], f32, tag="sA", name="sA")
            nc.scalar.copy(out=sA, in_=psA)
            psAT = psT_p.tile([B, 128], f32, tag="psVT", name="psAT")
            nc.tensor.transpose(psAT, sA, idn)
            uT = small.tile([B, 128], f32, tag="uT", name="uT")
            nc.scalar.copy(out=uT, in_=psAT)
            out_i, outT = squash(uT, 1.0 / R)

            # ---------- Two routing boundaries ----------
            for it in (1, 2):
                # --- V + delta ---
                for g in range(16):
                    x2k = big.tile([128, 2048], f32, tag="x2k", name="x2k")
                    nc.sync.dma_start(out=x2k, in_=x2d[:, 2048 * g:2048 * (g + 1)])
                    vs = [big.tile([128, 2048], f32, tag="vs", name=f"vs{p}")
                          for p in range(2)]
                    for t in range(4):
                        k = 4 * g + t
                        if vsp3:
                            wtkh = wt_p.tile([128, 512], bf16, tag="wtkh", name="wtkh")
                            nc.sync.dma_start(
                                out=wtkh,
                                in_=wth[:, :, 512 * k:512 * (k + 1)].rearrange(
                                    "c o n -> (c o) n"))
                            wtkl = wt_p.tile([128, 512], bf16, tag="wtkl", name="wtkl")
                            nc.sync.dma_start(
                                out=wtkl,
                                in_=wtl[:, :, 512 * k:512 * (k + 1)].rearrange(
                                    "c o n -> (c o) n"))
                        else:
                            wtk = wt_p.tile([128, 512], f32r if v32r else f32,
                                            tag="wtk", name="wtk")
                            eng = nc.gpsimd if v32r else nc.sync
                            eng.dma_start(
                                out=wtk,
                                in_=wt[:, :, 512 * k:512 * (k + 1)].rearrange(
                                    "c o n -> (c o) n"))
                        if vsp3:
                            oTh, oTl = outT
                            psV4 = [psV_p.tile([B, 512], f32, tag="psVT",
                                               name=f"psV4_{c}") for c in range(CLOC)]
                            for c4 in range(CLOC):
                                sl = slice(32 * c4, 32 * (c4 + 1))
                                tp = (32 * c4, 0)
                                nc.tensor.matmul(psV4[c4], oTh[sl, :], wtkh[sl, :],
                                                 start=True, stop=False,
                                                 tile_position=tp)
                                nc.tensor.matmul(psV4[c4], oTl[sl, :], wtkh[sl, :],
                                                 start=False, stop=False,
                                                 tile_position=tp)
                                nc.tensor.matmul(psV4[c4], oTh[sl, :], wtkl[sl, :],
                                                 start=False, stop=True,
                                                 tile_position=tp)
                            for c4 in range(CLOC):
                                pr, ce = divmod(c4, 2)
                                nc.scalar.copy(
                                    out=vs[pr][64 * ce:64 * (ce + 1),
                                               512 * t:512 * (t + 1)],
                                    in_=psV4[c4])
                        elif v32r:
                            psV4 = [psV_p.tile([B, 512], f32, tag="psVT",
                                               name=f"psV4_{c}") for c in range(CLOC)]
                            for c4 in range(CLOC):
                                nc.tensor.matmul(
                                    psV4[c4],
                                    outT[32 * c4:32 * (c4 + 1), :],
                                    wtk[32 * c4:32 * (c4 + 1), :],
                                    start=True, stop=True,
                                    tile_position=(32 * c4, 0))
                            for c4 in range(CLOC):
                                pr, ce = divmod(c4, 2)
                                nc.scalar.copy(
                                    out=vs[pr][64 * ce:64 * (ce + 1),
                                               512 * t:512 * (t + 1)],
                                    in_=psV4[c4])
                        else:
                            psV = [psV_p.tile([128, 512], f32, tag="psVT",
                                              name=f"psV{p}") for p in range(2)]
                            for c4 in range(CLOC):
                                pr, ce = divmod(c4, 2)
                                nc.tensor.matmul(
                                    psV[pr][64 * ce:64 * (ce + 1), :],
                                    outT[32 * c4:32 * (c4 + 1), :],
                                    wtk[32 * c4:32 * (c4 + 1), :],
                                    start=True, stop=True,
                                    tile_position=(32 * c4, 64 * ce))
                            for pr in range(2):
                                nc.scalar.copy(out=vs[pr][:, 512 * t:512 * (t + 1)],
                                               in_=psV[pr])
                    for pr in range(2):
                        nc.vector.tensor_mul(out=vs[pr], in0=vs[pr], in1=x2k)
                        if it == 1:
                            nc.vector.tensor_reduce(
                                out=lP[pr][:, 128 * g:128 * (g + 1)],
                                in_=vs[pr][:].rearrange("p (r j) -> p r j", j=CIN),
                                axis=AX.X, op=AL.add)
                        else:
                            dtmp = small.tile([128, 128], f32, tag="dtmp", name="dtmp")
                            nc.vector.tensor_reduce(
                                out=dtmp,
                                in_=vs[pr][:].rearrange("p (r j) -> p r j", j=CIN),
                                axis=AX.X, op=AL.add)
                            nc.vector.tensor_add(
                                out=lP[pr][:, 128 * g:128 * (g + 1)],
                                in0=lP[pr][:, 128 * g:128 * (g + 1)], in1=dtmp)

                # --- softmax pieces (unnormalized e + Z) ---
                zq = small.tile([B, CLOC], f32, tag="zq", name="zq")
                for pr in range(2):
                    m = small.tile([128, 1], f32, tag="m", name="m")
                    nc.vector.tensor_reduce(out=m, in_=lP[pr], axis=AX.X, op=AL.max)
                    mneg = small.tile([128, 1], f32, tag="mneg", name="mneg")
                    nc.vector.tensor_scalar_mul(out=mneg, in0=m, scalar1=-1.0)
                    eP = big.tile([128, R], f32, tag="e", name="eP")
                    Z = small.tile([128, 1], f32, tag="Z", name="Z")
                    nc.scalar.activation(out=eP, in_=lP[pr], func=AF.Exp,
                                         bias=mneg[:, 0:1], scale=1.0, accum_out=Z)
                    for ce in range(2):
                        nc.sync.dma_start(out=zq[:, 2 * pr + ce:2 * pr + ce + 1],
                                          in_=Z[64 * ce:64 * (ce + 1), 0:1])
                        for rb in range(R // 128):
                            psT2 = psT_p.tile([128, B], f32, tag="psVT", name="psT2")
                            nc.tensor.transpose(
                                psT2,
                                eP[64 * ce:64 * (ce + 1), 128 * rb:128 * (rb + 1)],
                                idn[64 * ce:64 * ce + 64, 64 * ce:64 * ce + 64])
                            nc.scalar.copy(out=p2T[:, 2 * pr + ce, rb, :], in_=psT2)
                rzq = small.tile([B, CLOC], f32, tag="rzq", name="rzq")
                nc.vector.reciprocal(out=rzq, in_=zq)

                # --- xe + s matmuls ---
                psS = [psacc_p.tile([B, 32], f32, tag="acc", name=f"psS{c}")
                       for c in range(CLOC)]
                for j in range(CIN):
                    xes = []
                    for c4 in range(CLOC):
                        xe = xe_p.tile([128, R // 128, B], f32, tag=f"xe{c4}",
                                       name=f"xe{c4}")
                        nc.vector.tensor_mul(
                            out=xe,
                            in0=xt2_sb[:, 16 * j:16 * (j + 1), :],
                            in1=p2T[:, c4, :, :])
                        xes.append(xe)
                    for t in range(R // 128):
                        k = 16 * j + t
                        wck = wcat_p.tile([128, 128], f32, tag="wck", name="wck")
                        nc.sync.dma_start(out=wck, in_=w2cat[128 * k:128 * (k + 1), :])
                        for c4 in range(CLOC):
                            nc.tensor.matmul(
                                psS[c4],
                                xes[c4][:, t, :],
                                wck[:, 32 * c4:32 * (c4 + 1)],
                                start=(k == 0), stop=(k == NK - 1))
                sS = small.tile([B, 128], f32, tag="sS", name="sS")
                for c4 in range(CLOC):
                    nc.scalar.activation(out=sS[:, 32 * c4:32 * (c4 + 1)],
                                         in_=psS[c4],
                                         func=AF.Copy, bias=0.0,
                                         scale=rzq[:, c4:c4 + 1])
                out_i, outT = squash(sS, 1.0)

            nc.sync.dma_start(out=out3[:], in_=out_i)

    nc.finalize()
    return nc


def _get_program():
    if "nc" not in _CACHE:
        _CACHE["nc"] = _build_program()
    return _CACHE["nc"]


def make_in_maps(x, route_weights):
    import ml_dtypes
    vsp3 = bool(int(os.environ.get("CAPS_VSPLIT3", "0")))
    x = np.ascontiguousarray(x, dtype=np.float32)
    W = np.ascontiguousarray(route_weights, dtype=np.float32)
    xt2 = np.ascontiguousarray(x.transpose(2, 1, 0).reshape(RJ, B))       # [(j,r),b]
    xnat = x.reshape(B, RJ)                                               # [b,(r,j)]
    x2d = np.ascontiguousarray(np.concatenate([xnat, xnat], axis=0))      # [128,(r,j)]
    ident = np.eye(128, dtype=np.float32)
    in_maps = []
    for core in range(NCORES):
        wc = W[CLOC * core:CLOC * (core + 1)]                             # [4,R,J,O]
        wtc = np.ascontiguousarray(
            wc.transpose(0, 3, 1, 2).reshape(CLOC, OUT, RJ))              # [c,o,(r,j)]
        m = {"w2cat": np.ascontiguousarray(
                wc.transpose(2, 1, 0, 3).reshape(RJ, CLOC * OUT)),        # [(j,r),(c,o)]
             "xt2": xt2, "wt": wtc, "x2d": x2d, "ident": ident}
        if vsp3:
            wth = wtc.astype(ml_dtypes.bfloat16)
            m["wth"] = wth
            m["wtl"] = (wtc - wth.astype(np.float32)).astype(ml_dtypes.bfloat16)
        in_maps.append(m)
    return in_maps


def kernel(x, route_weights):
    from concourse.bass_utils import run_bass_kernel_spmd

    in_maps = make_in_maps(x, route_weights)
    nc = _get_program()
    kw = {}
    if os.environ.get("CAPS_TRACE_DIR"):
        kw["tmpdir"] = os.environ["CAPS_TRACE_DIR"]
    res = run_bass_kernel_spmd(nc, in_maps, core_ids=list(range(NCORES)), **kw)
    if os.environ.get("CAPS_RESULT_STASH"):
        _CACHE["last_result"] = res

    out = np.empty((C, B, 1, 1, OUT), dtype=np.float32)
    for core in range(NCORES):
        o = res.results[core]["out3"].reshape(B, CLOC, OUT).transpose(1, 0, 2)
        out[CLOC * core:CLOC * (core + 1), :, 0, 0, :] = o
    return out



# revision 7
# speedup vs baseline: 3.0682x; 3.0682x over previous
"""CapsuleLayer dynamic-routing kernel for 8 Trainium2 NeuronCores (v2, fp16).

Problem: x [64,2048,16], route_weights [32,2048,16,32] ->
  3-iteration routing -> out [32,64,1,1,32] (fp32).

Sharding: capsules (C=32) split 4-per-core across 8 cores; x replicated.
All 16-bit data is fp16 (bf16 fails the 2e-2 gate: routing logits are
precision-sensitive; fp16 sim rel-err ~8e-3).

Per-core structure (c = 4 local capsules, b = 64, rj = 32768):
  phase A : psA[(c,o),b] += w2[k].T @ xt2[k]        (PE, 256 chunks, fp16)
  V step  : psU[(h,b), n] = oT_c.T @ wt[(c,o), n]   (PE, K=32 row-packed)
            vs = f16(psU)  (ACT)  ;  U = vs * x2h   (DVE 2x)
            delta = cascade-add over j (DVE), logits lP[c] += delta
  softmax : per-half max/exp (ACT accum Z), cross-half combine via small
            PE transposes -> alpha0/alpha1/rz in [b,c] layout
  s step  : xe_c = xt2 * eT_c (DVE 2x); psS[(c,o),(h,b)] += w2[k,c].T @ xe_c
            (PE col-group packed); s = (a0*s0T + a1*s1T)*rz; squash.
"""
import os
import numpy as np

C, B, R, CIN, OUT = 32, 64, 2048, 16, 32
NCORES = 8
CLOC = C // NCORES          # 4 capsules per core
RJ = R * CIN                # 32768  (j,r) / (r,j) linear size
NK = RJ // 128              # 256 chunks of 128
RH = RJ // 2                # 16384 cols per half in x2h / wt-half

_CACHE = {}


def _build_program():
    from contextlib import ExitStack
    import concourse.bass as bass
    import concourse.bacc as bacc
    import concourse.tile as tile
    from concourse import mybir

    f32 = mybir.dt.float32
    f16 = mybir.dt.float16
    AL = mybir.AluOpType
    AF = mybir.ActivationFunctionType
    AX = mybir.AxisListType

    nc = bacc.Bacc(None, target_bir_lowering=False,
                   detect_race_conditions=not bool(int(os.environ.get("CAPS_NO_RACE", "0"))))
    n_loops = int(os.environ.get("CAPS_LOOPS", "1"))

    # ---- DRAM I/O ----
    w2 = nc.dram_tensor("w2", [128, NK * 128], f16, kind="ExternalInput")  # [p,(k,co)]
    xt2 = nc.dram_tensor("xt2", [128, NK * B], f16, kind="ExternalInput")  # [p,(k,b)]
    x2h = nc.dram_tensor("x2h", [128, RH], f16, kind="ExternalInput")    # [(h,b),(r,j)/2]
    wt = nc.dram_tensor("wt", [CLOC, OUT, RJ], f16, kind="ExternalInput")  # [c,o,(r,j)]
    ident = nc.dram_tensor("ident", [128, 128], f32, kind="ExternalInput")
    out3 = nc.dram_tensor("out3", [B, 128], f32, kind="ExternalOutput")  # [b,(c,o)]

    with tile.TileContext(nc) as tc, ExitStack() as ctx:
        const = ctx.enter_context(tc.tile_pool(name="const", bufs=1))
        small = ctx.enter_context(tc.tile_pool(name="small", bufs=2))
        outp = ctx.enter_context(tc.tile_pool(name="outp", bufs=2))
        wtp_p = ctx.enter_context(tc.tile_pool(name="wtp", bufs=2))
        vs_p = ctx.enter_context(tc.tile_pool(name="vsp", bufs=2))
        cas_p = ctx.enter_context(tc.tile_pool(name="cas", bufs=2))
        xe_p = ctx.enter_context(tc.tile_pool(name="xep", bufs=2))
        eP_p = ctx.enter_context(tc.tile_pool(name="ep", bufs=1))
        psU_p = ctx.enter_context(tc.tile_pool(name="psU", bufs=2, space="PSUM"))
        psS0_p = ctx.enter_context(tc.tile_pool(name="psS0", bufs=1, space="PSUM"))
        psS1_p = ctx.enter_context(tc.tile_pool(name="psS1", bufs=1, space="PSUM"))
        psT_p = ctx.enter_context(tc.tile_pool(name="psT", bufs=2, space="PSUM"))

        idn = const.tile([128, 128], f32, tag="ident", name="idn")
        nc.sync.dma_start(out=idn, in_=ident[:])

        w2_sb = const.tile([128, NK, 128], f16, tag="w2sb", name="w2_sb")
        nc.sync.dma_start(out=w2_sb, in_=w2[:].rearrange("p (k co) -> p k co", k=NK))
        xt2_sb = const.tile([128, NK, B], f16, tag="xt2sb", name="xt2_sb")
        nc.sync.dma_start(out=xt2_sb, in_=xt2[:].rearrange("p (k b) -> p k b", k=NK))
        x2h_sb = const.tile([128, RH], f16, tag="x2h", name="x2h_sb")
        nc.sync.dma_start(out=x2h_sb, in_=x2h[:])

        # logits per capsule [(h,b)=128, r-in-half=1024]
        lP = [const.tile([128, R // 2], f32, tag=f"lP{c}", name=f"lP{c}")
              for c in range(CLOC)]
        # transposed probs [r%128, c, t=16, b]  (t: 0-7 half0 rb, 8-15 half1 rb)
        p2T = const.tile([128, CLOC, 16, B], f16, tag="p2T", name="p2T")

        def squash(u_bT, scale_pow):
            """u_bT [64,(4c,32o)] f32: s = u*scale_pow; out = s*sqrt(n2)/(n2+1).
            Returns (o_i [64,128] f32, oT [128,64] f16)."""
            sq = small.tile([B, 128], f32, tag="sq", name="sq")
            nc.vector.scalar_tensor_tensor(
                out=sq, in0=u_bT, scalar=float(scale_pow * scale_pow),
                in1=u_bT, op0=AL.mult, op1=AL.mult)
            n2 = small.tile([B, CLOC], f32, tag="n2", name="n2")
            nc.vector.tensor_reduce(
                out=n2, in_=sq[:].rearrange("b (c o) -> b c o", c=CLOC),
                axis=AX.X, op=AL.add)
            rt = small.tile([B, CLOC], f32, tag="rt", name="rt")
            nc.scalar.activation(out=rt, in_=n2, func=AF.Sqrt)
            dn = small.tile([B, CLOC], f32, tag="dn", name="dn")
            nc.vector.tensor_scalar_add(out=dn, in0=n2, scalar1=1.0)
            rc = small.tile([B, CLOC], f32, tag="rc", name="rc")
            nc.vector.reciprocal(out=rc, in_=dn)
            f = small.tile([B, CLOC], f32, tag="f", name="f")
            nc.vector.tensor_mul(out=f, in0=rt, in1=rc)
            f2 = small.tile([B, CLOC], f32, tag="f2", name="f2")
            nc.vector.tensor_scalar_mul(out=f2, in0=f, scalar1=float(scale_pow))
            o_i = outp.tile([B, 128], f32, tag="oi", name="oi")
            f2b = bass.AP(tensor=f2[:].tensor, offset=f2[:].offset,
                          ap=[f2[:].ap[0], f2[:].ap[1], [0, OUT]])
            nc.vector.tensor_tensor(
                out=o_i[:].rearrange("b (c o) -> b c o", c=CLOC),
                in0=u_bT[:].rearrange("b (c o) -> b c o", c=CLOC),
                in1=f2b, op=AL.mult)
            psOT = psT_p.tile([128, 128], f32, tag="psT", name="psOT")
            nc.tensor.transpose(psOT[:, 0:B], o_i, idn[0:B, 0:B])
            oT = outp.tile([128, B], f16, tag="oT", name="oT")
            nc.scalar.copy(out=oT, in_=psOT[:, 0:B])
            return o_i, oT

        for _loop in range(n_loops):
            # ---------- Phase A: s1 = (1/R) sum_(j,r) x W ----------
            psA = psS0_p.tile([128, B], f32, tag="psS0", name="psA")
            for k in range(NK):
                nc.tensor.matmul(psA, w2_sb[:, k, :], xt2_sb[:, k, :],
                                 start=(k == 0), stop=(k == NK - 1))
            sA = small.tile([128, B], f32, tag="sA", name="sA")
            nc.scalar.copy(out=sA, in_=psA)
            psAT = psT_p.tile([128, 128], f32, tag="psT", name="psAT")
            nc.tensor.transpose(psAT[0:B, :], sA, idn)
            uT = small.tile([B, 128], f32, tag="uT", name="uT")
            nc.scalar.copy(out=uT, in_=psAT[0:B, :])
            out_i, oT = squash(uT, 1.0 / R)

            # ---------- Two routing boundaries ----------
            for it in (1, 2):
                # --- V + delta (per 1024-col psU unit, both halves stacked) ---
                for u in range(16):
                    wtp = wtp_p.tile([128, 2, 1024], f16, tag="wtp", name="wtp")
                    nc.sync.dma_start(
                        out=wtp,
                        in_=wt[:].rearrange("c o (h n) -> (c o) h n", h=2)[
                            :, :, 1024 * u:1024 * (u + 1)])
                    for c in range(CLOC):
                        psU = psU_p.tile([128, 1024], f32, tag="psU", name="psU")
                        for i in range(2):
                            sl = slice(512 * i, 512 * (i + 1))
                            nc.tensor.matmul(
                                psU[0:64, sl], oT[32 * c:32 * (c + 1), :],
                                wtp[32 * c:32 * (c + 1), 0, sl],
                                start=True, stop=True, tile_position=(32 * c, 0))
                            nc.tensor.matmul(
                                psU[64:128, sl], oT[32 * c:32 * (c + 1), :],
                                wtp[32 * c:32 * (c + 1), 1, sl],
                                start=True, stop=True, tile_position=(32 * c, 64))
                        vs = vs_p.tile([128, 64, CIN], f16, tag="vs", name="vs")
                        nc.scalar.copy(
                            out=vs,
                            in_=psU[:].rearrange("p (r j) -> p r j", j=CIN))
                        nc.vector.tensor_tensor(
                            out=vs, in0=vs,
                            in1=x2h_sb[:, 1024 * u:1024 * (u + 1)].rearrange(
                                "p (r j) -> p r j", j=CIN),
                            op=AL.mult)
                        ca = cas_p.tile([128, 64, 8], f16, tag="ca", name="ca")
                        nc.vector.tensor_tensor(out=ca, in0=vs[:, :, 0:8],
                                                in1=vs[:, :, 8:16], op=AL.add)
                        cb = cas_p.tile([128, 64, 4], f16, tag="cb", name="cb")
                        nc.vector.tensor_tensor(out=cb, in0=ca[:, :, 0:4],
                                                in1=ca[:, :, 4:8], op=AL.add)
                        cc = cas_p.tile([128, 64, 2], f16, tag="cc", name="cc")
                        nc.vector.tensor_tensor(out=cc, in0=cb[:, :, 0:2],
                                                in1=cb[:, :, 2:4], op=AL.add)
                        if it == 1:
                            nc.vector.tensor_tensor(
                                out=lP[c][:, 64 * u:64 * (u + 1)],
                                in0=cc[:, :, 0], in1=cc[:, :, 1], op=AL.add)
                        else:
                            dt = small.tile([128, 64], f32, tag="dt", name="dt")
                            nc.vector.tensor_tensor(out=dt, in0=cc[:, :, 0],
                                                    in1=cc[:, :, 1], op=AL.add)
                            nc.vector.tensor_add(
                                out=lP[c][:, 64 * u:64 * (u + 1)],
                                in0=lP[c][:, 64 * u:64 * (u + 1)], in1=dt)

                # --- softmax pieces: per-half e, Z; cross-half alpha/Z combine ---
                mq = small.tile([128, 8], f32, tag="mq", name="mq")
                mn = small.tile([128, CLOC], f32, tag="mn", name="mn")
                for c in range(CLOC):
                    nc.vector.tensor_reduce(out=mq[:, c:c + 1], in_=lP[c],
                                            axis=AX.X, op=AL.max)
                    nc.vector.tensor_scalar_mul(out=mn[:, c:c + 1],
                                                in0=mq[:, c:c + 1], scalar1=-1.0)
                    eP = eP_p.tile([128, R // 2], f32, tag="eP", name="eP")
                    nc.scalar.activation(out=eP, in_=lP[c], func=AF.Exp,
                                         bias=mn[:, c:c + 1], scale=1.0,
                                         accum_out=mq[:, 4 + c:5 + c])
                    for rb in range(8):
                        psT2 = psT_p.tile([128, 128], f32, tag="psT", name="psT2")
                        nc.tensor.transpose(
                            psT2, eP[:, 128 * rb:128 * (rb + 1)], idn)
                        nc.scalar.copy(out=p2T[:, c, rb, :], in_=psT2[:, 0:64])
                        nc.scalar.copy(out=p2T[:, c, 8 + rb, :], in_=psT2[:, 64:128])
                # cross-half combine of m and Z via transposes to [4,128]
                psM = psT_p.tile([128, 128], f32, tag="psT", name="psM")
                nc.tensor.transpose(psM[0:4, :], mq[:, 0:4], idn)
                mT = small.tile([CLOC, 128], f32, tag="mT", name="mT")
                nc.scalar.copy(out=mT, in_=psM[0:4, :])
                psZ = psT_p.tile([128, 128], f32, tag="psT", name="psZ")
                nc.tensor.transpose(psZ[0:4, :], mq[:, 4:8], idn)
                zT = small.tile([CLOC, 128], f32, tag="zT", name="zT")
                nc.scalar.copy(out=zT, in_=psZ[0:4, :])
                mc = small.tile([CLOC, 64], f32, tag="mc", name="mc")
                nc.vector.tensor_tensor(out=mc, in0=mT[:, 0:64],
                                        in1=mT[:, 64:128], op=AL.max)
                aa = small.tile([CLOC, 3, 64], f32, tag="aa", name="aa")
                for h in range(2):
                    dm = small.tile([CLOC, 64], f32, tag="dm", name="dm")
                    nc.vector.tensor_sub(out=dm, in0=mT[:, 64 * h:64 * (h + 1)],
                                         in1=mc)
                    nc.scalar.activation(out=aa[:, h, :], in_=dm, func=AF.Exp)
                z0 = small.tile([CLOC, 64], f32, tag="z0", name="z0")
                nc.vector.tensor_mul(out=z0, in0=zT[:, 0:64], in1=aa[:, 0, :])
                z1 = small.tile([CLOC, 64], f32, tag="z1", name="z1")
                nc.vector.tensor_mul(out=z1, in0=zT[:, 64:128], in1=aa[:, 1, :])
                zc = small.tile([CLOC, 64], f32, tag="zc", name="zc")
                nc.vector.tensor_add(out=zc, in0=z0, in1=z1)
                nc.vector.reciprocal(out=aa[:, 2, :], in_=zc)
                # transpose [4,3*64] -> per-b [64, (3,4)] in one shot:
                # aa rows=c(4), cols=(h/rz 3, b 64): transpose -> [(3,64)?? no:
                # transpose each [4,64] slice separately into ab [64, 3, 4]
                ab = small.tile([64, 3, CLOC], f32, tag="ab", name="ab")
                for s3 in range(3):
                    psa = psT_p.tile([128, 128], f32, tag="psT", name="psa")
                    nc.tensor.transpose(psa[0:64, 0:4], aa[:, s3, :], idn[0:4, 0:4])
                    nc.scalar.copy(out=ab[:, s3, :], in_=psa[0:64, 0:4])

                # --- xe + s matmuls (psS cols: [0:64] half0, [64:128] half1) ---
                psS = [psS0_p.tile([128, B], f32, tag="psS0", name="psS0i"),
                       psS1_p.tile([128, B], f32, tag="psS1", name="psS1i")]
                nc.vector.memset(psS[0][:], 0.0)
                nc.vector.memset(psS[1][:], 0.0)
                for j in range(CIN):
                    xes = []
                    for c in range(CLOC):
                        xe = xe_p.tile([128, 16, B], f16, tag=f"xe{c}", name=f"xe{c}")
                        nc.vector.tensor_tensor(
                            out=xe, in0=xt2_sb[:, 16 * j:16 * (j + 1), :],
                            in1=p2T[:, c, :, :], op=AL.mult)
                        xes.append(xe)
                    for t in range(16):
                        k = 16 * j + t
                        h = t // 8
                        for c in range(CLOC):
                            nc.tensor.matmul(
                                psS[h][32 * c:32 * (c + 1), :],
                                w2_sb[:, k, 32 * c:32 * (c + 1)],
                                xes[c][:, t, :],
                                start=False, stop=(k == 240 + 8 * h + 7),
                                tile_position=(0, 32 * c), skip_group_check=True)
                # s = (a0*s0T + a1*s1T) * rz ; squash
                sc = small.tile([128, 2, B], f32, tag="sc", name="sc")
                nc.scalar.copy(out=sc[:, 0, :], in_=psS[0])
                nc.scalar.copy(out=sc[:, 1, :], in_=psS[1])
                psH = psT_p.tile([128, 128], f32, tag="psT", name="psH")
                nc.tensor.transpose(psH[0:64, :], sc[:, 0, :], idn)
                s0T = small.tile([64, 128], f32, tag="s0T", name="s0T")
                nc.scalar.copy(out=s0T, in_=psH[0:64, :])
                psH2 = psT_p.tile([128, 128], f32, tag="psT", name="psH2")
                nc.tensor.transpose(psH2[0:64, :], sc[:, 1, :], idn)
                s1T = small.tile([64, 128], f32, tag="s1T", name="s1T")
                nc.scalar.copy(out=s1T, in_=psH2[0:64, :])

                def bcast(col):
                    a = ab[:, col, :]
                    return bass.AP(tensor=a.tensor, offset=a.offset,
                                   ap=[a.ap[0], a.ap[1], [0, OUT]])
                u0 = small.tile([64, 128], f32, tag="u0", name="u0")
                nc.vector.tensor_tensor(
                    out=u0[:].rearrange("b (c o) -> b c o", c=CLOC),
                    in0=s0T[:].rearrange("b (c o) -> b c o", c=CLOC),
                    in1=bcast(0), op=AL.mult)
                u1 = small.tile([64, 128], f32, tag="u1", name="u1")
                nc.vector.tensor_tensor(
                    out=u1[:].rearrange("b (c o) -> b c o", c=CLOC),
                    in0=s1T[:].rearrange("b (c o) -> b c o", c=CLOC),
                    in1=bcast(1), op=AL.mult)
                us = small.tile([64, 128], f32, tag="us", name="us")
                nc.vector.tensor_add(out=us, in0=u0, in1=u1)
                uF = small.tile([64, 128], f32, tag="uF", name="uF")
                nc.vector.tensor_tensor(
                    out=uF[:].rearrange("b (c o) -> b c o", c=CLOC),
                    in0=us[:].rearrange("b (c o) -> b c o", c=CLOC),
                    in1=bcast(2), op=AL.mult)
                out_i, oT = squash(uF, 1.0)

            nc.sync.dma_start(out=out3[:], in_=out_i)

    nc.finalize()
    return nc


def _get_program():
    if "nc" not in _CACHE:
        _CACHE["nc"] = _build_program()
    return _CACHE["nc"]


def make_in_maps(x, route_weights):
    x = np.ascontiguousarray(x, dtype=np.float32)
    W = np.ascontiguousarray(route_weights, dtype=np.float32)
    xt2 = np.ascontiguousarray(
        x.transpose(2, 1, 0).reshape(NK, 128, B).transpose(1, 0, 2)
        .reshape(128, NK * B)).astype(np.float16)                 # [p,(k,b)]
    xnat = x.reshape(B, RJ)                                       # [b,(r,j)]
    x2h = np.ascontiguousarray(
        np.concatenate([xnat[:, :RH], xnat[:, RH:]], axis=0)).astype(np.float16)
    ident = np.eye(128, dtype=np.float32)
    in_maps = []
    for core in range(NCORES):
        wc = W[CLOC * core:CLOC * (core + 1)]                     # [4,R,J,O]
        m = {"w2": np.ascontiguousarray(
                wc.transpose(2, 1, 0, 3).reshape(NK, 128, CLOC * OUT)
                .transpose(1, 0, 2).reshape(128, NK * 128)).astype(np.float16),
             "wt": np.ascontiguousarray(
                wc.transpose(0, 3, 1, 2).reshape(CLOC, OUT, RJ)).astype(np.float16),
             "xt2": xt2, "x2h": x2h, "ident": ident}
        in_maps.append(m)
    return in_maps


def kernel(x, route_weights):
    from concourse.bass_utils import run_bass_kernel_spmd

    in_maps = make_in_maps(x, route_weights)
    nc = _get_program()
    kw = {}
    if os.environ.get("CAPS_TRACE_DIR"):
        kw["tmpdir"] = os.environ["CAPS_TRACE_DIR"]
    res = run_bass_kernel_spmd(nc, in_maps, core_ids=list(range(NCORES)), **kw)
    if os.environ.get("CAPS_RESULT_STASH"):
        _CACHE["last_result"] = res

    out = np.empty((C, B, 1, 1, OUT), dtype=np.float32)
    for core in range(NCORES):
        o = res.results[core]["out3"].reshape(B, CLOC, OUT).transpose(1, 0, 2)
        out[CLOC * core:CLOC * (core + 1), :, 0, 0, :] = o
    return out
